# revision 10
# baseline (speedup 1.0000x reference)
"""v7: pure-elementwise Trainium kernel for the coupled-pendulum ODE.

Math (host-validated): order-6 explicit Stormer-Cowell multistep on
  d2theta/dtau2 = cp*lap_ring(theta) - sin(2pi theta)/(2pi)
in turns/tau units (tau = omega0*t, theta in turns), cp = coupling/omega0^2,
with a 4-step RKN4 startup. One force eval per Cowell step.

Chaos note: the system amplifies per-step perturbations ~1e3 over tau=20,
so EVERYTHING stays fp32 — fp32r/bf16 matmuls fail the 2e-2 gate (measured
via host-model bisection). Hence no PE: the ring Laplacian is computed on
the free axis via shifted-AP difference ops (layout: batch rows on the 128
partitions, ring position on the 512-wide free axis; per core 128 rows =
exactly the 1024/8 batch slice, no host transposes).

a-hat form: ah = 2pi*cp*lap(theta) + sin(-2pi*wrap(theta)), so accel
A = ah/(2pi); every combine is then a scalar_tensor_tensor (stt) op and
scale constants fold into fp32 immediates downstream.

Engines per Cowell step (ops split into 2 column-streams of [128,256]
to hide the serial chain; all tiles are single [128,512] so ring wraps
are in-tile single-column fixups):
  DVE - TURNS_WRAP, D = diff(theta), lap = diff(D), theta update, fixups,
        history Horner head (s1, s2)
  ACT - Sin
  GPS - ah combine, Horner tail (s3, s4), Z update   (all stt, 0.6 eff)
  PE/DMA - idle (DMA only for initial load / final store)
"""

import math

import numpy as np

import concourse.bacc as bacc
import concourse.bass as bass
import concourse.dve_ops as dve_ops
import concourse.mybir as mybir
import concourse.tile as tile
from concourse.bass_utils import run_bass_kernel_spmd
from concourse.dve_spec import C0, C1, C2, Spec, Src0, Src1, _has_src1, lower
from concourse.dve_uop import DveOpSpec

F32 = mybir.dt.float32
AF = mybir.ActivationFunctionType
OP = mybir.AluOpType

N_CORES = 8
B, N = 1024, 512
PB = B // N_CORES            # 128 batch rows per core = partition dim

NSTEPS = 160                 # SC6 steps (incl. 4 RKN4 startup steps)
T_END = 2.0
TWO_PI = 2 * math.pi
MAGIC = 12582912.0           # 1.5 * 2**23: fp32 round-to-int trick

# order-6 explicit Stormer-Cowell: theta_{n+1} = 2 theta_n - theta_{n-1}
#   + h^2 * sum_j BETA[j] * A_{n-j}   (host-validated)
BETA = [299.0 / 240.0, -176.0 / 240.0, 194.0 / 240.0,
        -96.0 / 240.0, 19.0 / 240.0]
NHIST = len(BETA)
NSTART = NHIST - 1           # RKN4 startup steps


def _register_custom_op(name, body, reference):
    for op in dve_ops.OPS:
        if op.name == name:
            return op
    idx = dve_ops._CUSTOM_DVE_ROW_BASE + len(dve_ops.OPS)
    assert idx < 0x20
    spec = Spec(body=body, reference=reference)
    shas = {}
    for ver in ("v3", "v4"):
        try:
            uops = lower(spec, ver=ver)
            tmp = DveOpSpec(name=name, opcode=idx, uops=uops,
                            rd1_en=_has_src1(spec))
            shas[ver] = tmp.sha(ver)
        except Exception:
            pass
    op = dve_ops.DveOp(name, spec, subdim=False, uops_sha=shas)
    dve_ops.OPS.append(op)
    dve_ops._SUB_OPCODE_FOR_NAME[name] = idx
    dve_ops.CUSTOM_DVE_SPECS[name] = spec
    return op


def _f32(v):
    return np.float32(v)


_tw_z = Src0 * C0 + Src1 * C1
TURNS_WRAP = _register_custom_op(
    "TURNS_WRAP_ANT",
    _tw_z - ((_tw_z + C2) - C2),
    lambda in0, in1, s0, s1, imm2: (
        lambda z: z - ((z + _f32(imm2)) - _f32(imm2)))(
        (in0.astype(np.float32) * _f32(s0)
         + in1.astype(np.float32) * _f32(s1)).astype(np.float32)),
)


class _Steps:
    """Emits the per-step instruction streams for one core."""

    def __init__(self, nc, pool, h, cp):
        self.nc = nc
        self.pool = pool
        self.h = h
        self.cp = cp
        self.g = float(2 * math.pi * cp)      # ah = g*lap + ns
        # 2 column streams; slice bounds
        self.sl = [(0, 256), (256, 512)]

    def tl(self, name, tag=None):
        return self.pool.tile([128, N], F32, name=name, tag=tag or name)

    # --- elementwise helpers ---
    # scalar_tensor_tensor is DVE-only on TRN2 (Pool fails the ISA engine
    # check for TensorScalarPtr); GPSIMD takes the plain adds/subs.
    def stt(self, eng, out, in0, s, in1):
        # out = s*in0 + in1, emitted per stream (always DVE)
        for c0, c1 in self.sl:
            self.nc.vector.scalar_tensor_tensor(
                out[:, c0:c1], in0[:, c0:c1], float(s), in1[:, c0:c1],
                OP.mult, OP.add)

    def stt_full(self, out, in0, s, in1):
        # single full-width instruction (off-critical-path combines)
        self.nc.vector.scalar_tensor_tensor(out[:], in0[:], float(s),
                                            in1[:], OP.mult, OP.add)

    def add_gps(self, out, in0, in1):
        # out = in0 + in1 on GPSIMD, per stream
        for c0, c1 in self.sl:
            self.nc.gpsimd.tensor_tensor(out[:, c0:c1], in0[:, c0:c1],
                                         in1[:, c0:c1], OP.add)

    def wrap2(self, out, in0, s0, in1, s1):
        for c0, c1 in self.sl:
            self.nc.vector._custom_dve(
                TURNS_WRAP, out=out[:, c0:c1], in0=in0[:, c0:c1],
                in1=in1[:, c0:c1], s0=float(s0), s1=float(s1), imm2=MAGIC)

    def sin(self, out, in_):
        for c0, c1 in self.sl:
            self.nc.scalar.activation(out[:, c0:c1], in_[:, c0:c1], AF.Sin,
                                      bias=0.0, scale=-TWO_PI)

    def lap(self, q, dt_, lp):
        """lp = ring laplacian of q via two difference passes.
        D[j] = q[j+1]-q[j] (j=511 wraps); lap[j] = D[j]-D[j-1] (j=0 wraps).
        Main ops on GPSIMD (sub); single-col wrap fixups on DVE."""
        nc = self.nc
        # D: stream0 j=0..255 affine; stream1 j=256..510 affine + j=511 tiny
        nc.gpsimd.tensor_tensor(dt_[:, 0:256], q[:, 1:257], q[:, 0:256],
                                OP.subtract)
        nc.gpsimd.tensor_tensor(dt_[:, 256:511], q[:, 257:512],
                                q[:, 256:511], OP.subtract)
        nc.vector.tensor_tensor(dt_[:, 511:512], q[:, 0:1], q[:, 511:512],
                                OP.subtract)
        # lap: stream0 j=1..255 affine + j=0 tiny; stream1 j=256..511 affine
        nc.gpsimd.tensor_tensor(lp[:, 1:256], dt_[:, 1:256], dt_[:, 0:255],
                                OP.subtract)
        nc.vector.tensor_tensor(lp[:, 0:1], dt_[:, 0:1], dt_[:, 511:512],
                                OP.subtract)
        nc.gpsimd.tensor_tensor(lp[:, 256:512], dt_[:, 256:512],
                                dt_[:, 255:511], OP.subtract)

    def ah_eval(self, q, ah, tag):
        """ah = g*lap(q) + sin(-2pi*wrap(q)); returns nothing (writes ah)."""
        dt_ = self.tl("D", f"D_{tag}")
        lp = self.tl("lp", f"lp_{tag}")
        w = self.tl("w", f"w_{tag}")
        ns = self.tl("ns", f"ns_{tag}")
        self.lap(q, dt_, lp)
        self.wrap2(w, q, 1.0, q, 0.0)
        self.sin(ns, w)
        self.stt("dve", ah, lp, self.g, ns)

    def ah_eval_fused(self, a, sa, b, ah, q_out, tag):
        """Position q = sa*a + b; emit q (for lap) plus fused wrap, then
        ah = g*lap(q) + ns.  q_out must be a fresh tile."""
        dt_ = self.tl("D", f"D_{tag}")
        lp = self.tl("lp", f"lp_{tag}")
        w = self.tl("w", f"w_{tag}")
        ns = self.tl("ns", f"ns_{tag}")
        self.stt("dve", q_out, a, sa, b)      # q materialized for shifts
        self.wrap2(w, a, sa, b, 1.0)          # frac(sa*a + b) w/o waiting q
        self.sin(ns, w)
        self.lap(q_out, dt_, lp)
        self.stt("dve", ah, lp, self.g, ns)

    def rkn4_step(self, th, u, th_new, u_new, ah1_out, tag):
        """One RKN4 step; ah1_out receives ah(theta_n) for SC history."""
        h, g = self.h, self.g
        kap = h * h / 8.0                      # a1s = (kap/2pi) * ah1
        lam = h * h / 2.0                      # a2s = (lam/2pi) * ah2
        self.ah_eval(th, ah1_out, f"e1_{tag}")
        p2a = self.tl("p2a", f"p2a_{tag}")
        self.stt("dve", p2a, u, h / 2.0, th)
        q2 = self.tl("q2", f"q2_{tag}")
        ah2 = self.tl("ah2", f"ah2_{tag}")
        self.ah_eval_fused(ah1_out, kap / TWO_PI, p2a, ah2, q2,
                           f"e2_{tag}")
        t = self.tl("t", f"t_{tag}")
        self.stt("dve", t, u, h, th)
        q3 = self.tl("q3", f"q3_{tag}")
        ah3 = self.tl("ah3", f"ah3_{tag}")
        self.ah_eval_fused(ah2, lam / TWO_PI, t, ah3, q3, f"e3_{tag}")
        # theta' = t + (kap/2pi)*(4/3)*(ah1 + 2 ah2)
        g2 = self.tl("g2", f"g2_{tag}")
        self.stt("dve", g2, ah2, 2.0, ah1_out)
        self.stt("dve", th_new, g2, (4.0 / 3.0) * kap / TWO_PI, t)
        # u' = u + (h/(12pi)) * (ah1 + 4 ah2 + ah3)
        z1 = self.tl("z1", f"z1_{tag}")
        self.stt("dve", z1, ah2, 4.0, ah1_out)
        z2 = self.tl("z2", f"z2_{tag}")
        self.add_gps(z2, z1, ah3)
        self.stt("dve", u_new, z2, h / (12.0 * math.pi), u)

    def sc6_step(self, th, Z, th_new, Z_new, hist, ah_out, tag):
        """One Cowell step. hist = [ah_{n-1},...,ah_{n-4}] (newest first);
        ah_out receives ah(theta_n).
        theta' = theta + Z'; Z' = Z + (h^2/2pi)*sum beta_j ah_{n-j}."""
        h = self.h
        # Horner over history (independent of this step's eval): emit first,
        # as single full-width DVE ops (off the critical path)
        s1 = self.tl("s1", f"s1_{tag}")
        s2 = self.tl("s2", f"s2_{tag}")
        s3 = self.tl("s3", f"s3_{tag}")
        s4 = self.tl("s4", f"s4_{tag}")
        self.stt_full(s1, hist[3], BETA[4] / BETA[3], hist[2])
        self.stt_full(s2, s1, BETA[3] / BETA[2], hist[1])
        self.stt_full(s3, s2, BETA[2] / BETA[1], hist[0])
        self.ah_eval(th, ah_out, f"cw_{tag}")
        self.stt("dve", s4, s3, BETA[1] / BETA[0], ah_out)
        self.stt("dve", Z_new, s4, BETA[0] * h * h / TWO_PI, Z)
        self.add_gps(th_new, th, Z_new)


def _build(nsteps: int, omega0: float, coupling: float) -> bass.Bass:
    tau_end = omega0 * T_END
    h = tau_end / nsteps
    cp = coupling / (omega0 * omega0)

    nc = bacc.Bacc("TRN2", target_bir_lowering=False, debug=False,
                   num_devices=N_CORES)
    x_in = nc.dram_tensor("x", [128, N], F32, kind="ExternalInput")
    out = nc.dram_tensor("out", [128, N], F32, kind="ExternalOutput")

    with tile.TileContext(nc) as tc:
        with (
            tc.tile_pool(name="state", bufs=1) as state,
            tc.tile_pool(name="tmp", bufs=2) as tmp,
        ):
            st = _Steps(nc, tmp, h, cp)

            xs = state.tile([128, N], F32, name="xs")
            nc.gpsimd.dma_start(xs[:], x_in[:])
            th = [state.tile([128, N], F32, name=f"th{i}") for i in range(2)]
            u = [state.tile([128, N], F32, name=f"u{i}") for i in range(2)]
            Z = [state.tile([128, N], F32, name=f"Z{i}") for i in range(2)]
            hist = [state.tile([128, N], F32, name=f"ah{i}")
                    for i in range(NHIST)]

            # init: theta = x - 0.5 (turns); u = 0
            nc.scalar.activation(th[0][:], xs[:], AF.Copy, bias=-0.5,
                                 scale=1.0)
            nc.vector.memset(u[0][:], 0.0)

            n_sc = nsteps - NSTART
            assert n_sc >= 1
            # startup: RKN4 steps, saving ah(theta_n) into history
            # hist order after startup (newest first for step NSTART):
            #   [ah_{NSTART-1}, ..., ah_0]
            cur, nxt = 0, 1
            prev_th = None
            for n in range(NSTART):
                st.rkn4_step(th[cur], u[cur], th[nxt], u[nxt],
                             hist[NSTART - 1 - n], "su")
                prev_th = th[cur]
                cur, nxt = nxt, cur
            # Z = theta_NSTART - theta_{NSTART-1}
            nc.vector.tensor_tensor(Z[0][:], th[cur][:], prev_th[:],
                                    OP.subtract)

            zc, zn = 0, 1
            hlist = list(hist[:NSTART])       # newest first
            for n in range(n_sc):
                # write into the slot that just fell out of the history
                ah_new = hist[(NHIST - 1 - (n % NHIST)) % NHIST]
                st.sc6_step(th[cur], Z[zc], th[nxt], Z[zn], hlist, ah_new,
                            "cw")
                hlist = [ah_new] + hlist[:NSTART - 1]
                cur, nxt = nxt, cur
                zc, zn = zn, zc

            # out = 2pi * theta  (radians)
            rad = tmp.tile([128, N], F32, name="rad", tag="rad")
            nc.scalar.activation(rad[:], th[cur][:], AF.Copy, bias=0.0,
                                 scale=TWO_PI)
            nc.gpsimd.dma_start(out[:], rad[:])

    nc.compile()
    return nc


_CACHE: dict = {}


def kernel(x, omega0, coupling, nsteps: int = None):
    x = np.ascontiguousarray(np.asarray(x, dtype=np.float32))
    om = float(np.asarray(omega0, dtype=np.float64))
    cp = float(np.asarray(coupling, dtype=np.float64))
    if nsteps is None:
        nsteps = NSTEPS
    key = (nsteps, om, cp)
    if key not in _CACHE:
        _CACHE[key] = _build(nsteps, om, cp)
    nc = _CACHE[key]

    in_maps = [{"x": x[i * PB:(i + 1) * PB]} for i in range(N_CORES)]
    res = run_bass_kernel_spmd(nc, in_maps, list(range(N_CORES)))
    return np.concatenate([r["out"] for r in res.results],
                          axis=0).astype(np.float32)


# revision 14
# speedup vs baseline: 1.5073x; 1.5073x over previous
"""v7: pure-elementwise Trainium kernel for the coupled-pendulum ODE.

Math (host-validated): order-6 explicit Stormer-Cowell multistep on
  d2theta/dtau2 = cp*lap_ring(theta) - sin(2pi theta)/(2pi)
in turns/tau units (tau = omega0*t, theta in turns), cp = coupling/omega0^2,
with a 4-step RKN4 startup. One force eval per Cowell step.

Chaos note: the system amplifies per-step perturbations ~1e3 over tau=20,
so EVERYTHING stays fp32 — fp32r/bf16 matmuls fail the 2e-2 gate (measured
via host-model bisection). Hence no PE: the ring Laplacian is computed on
the free axis via shifted-AP difference ops (layout: batch rows on the 128
partitions, ring position on the 512-wide free axis; per core 128 rows =
exactly the 1024/8 batch slice, no host transposes).

a-hat form: ah = 2pi*cp*lap(theta) + sin(-2pi*wrap(theta)), so accel
A = ah/(2pi); every combine is then a scalar_tensor_tensor (stt) op and
scale constants fold into fp32 immediates downstream.

Engines per Cowell step (ops split into 2 column-streams of [128,256]
to hide the serial chain; all tiles are single [128,512] so ring wraps
are in-tile single-column fixups):
  DVE - TURNS_WRAP, D = diff(theta), lap = diff(D), theta update, fixups,
        history Horner head (s1, s2)
  ACT - Sin
  GPS - ah combine, Horner tail (s3, s4), Z update   (all stt, 0.6 eff)
  PE/DMA - idle (DMA only for initial load / final store)
"""

import math

import numpy as np

import concourse.bacc as bacc
import concourse.bass as bass
import concourse.dve_ops as dve_ops
import concourse.mybir as mybir
import concourse.tile as tile
from concourse.bass_utils import run_bass_kernel_spmd
from concourse.dve_spec import C0, C1, C2, Spec, Src0, Src1, _has_src1, lower
from concourse.dve_uop import DveOpSpec

F32 = mybir.dt.float32
AF = mybir.ActivationFunctionType
OP = mybir.AluOpType

N_CORES = 8
B, N = 1024, 512
PB = B // N_CORES            # 128 batch rows per core = partition dim

NSTEPS = 160                 # SC6 steps (incl. 4 RKN4 startup steps)
GPS_OFF = True               # A/B: route Pool (GPSIMD) ops to DVE instead
T_END = 2.0
TWO_PI = 2 * math.pi
MAGIC = 12582912.0           # 1.5 * 2**23: fp32 round-to-int trick

# order-6 explicit Stormer-Cowell: theta_{n+1} = 2 theta_n - theta_{n-1}
#   + h^2 * sum_j BETA[j] * A_{n-j}   (host-validated)
BETA = [299.0 / 240.0, -176.0 / 240.0, 194.0 / 240.0,
        -96.0 / 240.0, 19.0 / 240.0]
NHIST = len(BETA)
NSTART = NHIST - 1           # RKN4 startup steps


def _register_custom_op(name, body, reference):
    for op in dve_ops.OPS:
        if op.name == name:
            return op
    idx = dve_ops._CUSTOM_DVE_ROW_BASE + len(dve_ops.OPS)
    assert idx < 0x20
    spec = Spec(body=body, reference=reference)
    shas = {}
    for ver in ("v3", "v4"):
        try:
            uops = lower(spec, ver=ver)
            tmp = DveOpSpec(name=name, opcode=idx, uops=uops,
                            rd1_en=_has_src1(spec))
            shas[ver] = tmp.sha(ver)
        except Exception:
            pass
    op = dve_ops.DveOp(name, spec, subdim=False, uops_sha=shas)
    dve_ops.OPS.append(op)
    dve_ops._SUB_OPCODE_FOR_NAME[name] = idx
    dve_ops.CUSTOM_DVE_SPECS[name] = spec
    return op


def _f32(v):
    return np.float32(v)


_tw_z = Src0 * C0 + Src1 * C1
TURNS_WRAP = _register_custom_op(
    "TURNS_WRAP_ANT",
    _tw_z - ((_tw_z + C2) - C2),
    lambda in0, in1, s0, s1, imm2: (
        lambda z: z - ((z + _f32(imm2)) - _f32(imm2)))(
        (in0.astype(np.float32) * _f32(s0)
         + in1.astype(np.float32) * _f32(s1)).astype(np.float32)),
)


class _Steps:
    """Emits the per-step instruction streams for one core."""

    def __init__(self, nc, pool, h, cp):
        self.nc = nc
        self.pool = pool
        self.h = h
        self.cp = cp
        self.g = float(2 * math.pi * cp)      # ah = g*lap + ns
        # 2 column streams; slice bounds
        self.sl = [(0, 256), (256, 512)]

    def tl(self, name, tag=None):
        return self.pool.tile([128, N], F32, name=name, tag=tag or name)

    # --- elementwise helpers ---
    # scalar_tensor_tensor is DVE-only on TRN2 (Pool fails the ISA engine
    # check for TensorScalarPtr); GPSIMD takes the plain adds/subs.
    def stt(self, eng, out, in0, s, in1):
        # out = s*in0 + in1, emitted per stream (always DVE)
        for c0, c1 in self.sl:
            self.nc.vector.scalar_tensor_tensor(
                out[:, c0:c1], in0[:, c0:c1], float(s), in1[:, c0:c1],
                OP.mult, OP.add)

    def stt_full(self, out, in0, s, in1):
        # single full-width instruction (off-critical-path combines)
        self.nc.vector.scalar_tensor_tensor(out[:], in0[:], float(s),
                                            in1[:], OP.mult, OP.add)

    def add_gps(self, out, in0, in1):
        # out = in0 + in1 on GPSIMD (or DVE full-width when GPS_OFF)
        if GPS_OFF:
            self.nc.vector.tensor_tensor(out[:], in0[:], in1[:], OP.add)
            return
        for c0, c1 in self.sl:
            self.nc.gpsimd.tensor_tensor(out[:, c0:c1], in0[:, c0:c1],
                                         in1[:, c0:c1], OP.add)

    def wrap2(self, out, in0, s0, in1, s1):
        for c0, c1 in self.sl:
            self.nc.vector._custom_dve(
                TURNS_WRAP, out=out[:, c0:c1], in0=in0[:, c0:c1],
                in1=in1[:, c0:c1], s0=float(s0), s1=float(s1), imm2=MAGIC)

    def sin(self, out, in_):
        for c0, c1 in self.sl:
            self.nc.scalar.activation(out[:, c0:c1], in_[:, c0:c1], AF.Sin,
                                      bias=0.0, scale=-TWO_PI)

    def lap(self, q, dt_, lp):
        """lp = ring laplacian of q via two difference passes.
        D[j] = q[j+1]-q[j] (j=511 wraps); lap[j] = D[j]-D[j-1] (j=0 wraps).
        Main ops on GPSIMD (sub) or merged full-width on DVE (GPS_OFF);
        single-col wrap fixups on DVE."""
        nc = self.nc
        if GPS_OFF:
            nc.vector.tensor_tensor(dt_[:, 0:511], q[:, 1:512], q[:, 0:511],
                                    OP.subtract)
            nc.vector.tensor_tensor(dt_[:, 511:512], q[:, 0:1],
                                    q[:, 511:512], OP.subtract)
            nc.vector.tensor_tensor(lp[:, 1:512], dt_[:, 1:512],
                                    dt_[:, 0:511], OP.subtract)
            nc.vector.tensor_tensor(lp[:, 0:1], dt_[:, 0:1], dt_[:, 511:512],
                                    OP.subtract)
            return
        # D: stream0 j=0..255 affine; stream1 j=256..510 affine + j=511 tiny
        nc.gpsimd.tensor_tensor(dt_[:, 0:256], q[:, 1:257], q[:, 0:256],
                                OP.subtract)
        nc.gpsimd.tensor_tensor(dt_[:, 256:511], q[:, 257:512],
                                q[:, 256:511], OP.subtract)
        nc.vector.tensor_tensor(dt_[:, 511:512], q[:, 0:1], q[:, 511:512],
                                OP.subtract)
        # lap: stream0 j=1..255 affine + j=0 tiny; stream1 j=256..511 affine
        nc.gpsimd.tensor_tensor(lp[:, 1:256], dt_[:, 1:256], dt_[:, 0:255],
                                OP.subtract)
        nc.vector.tensor_tensor(lp[:, 0:1], dt_[:, 0:1], dt_[:, 511:512],
                                OP.subtract)
        nc.gpsimd.tensor_tensor(lp[:, 256:512], dt_[:, 256:512],
                                dt_[:, 255:511], OP.subtract)

    def ah_eval(self, q, ah, tag):
        """ah = g*lap(q) + sin(-2pi*wrap(q)); returns nothing (writes ah)."""
        dt_ = self.tl("D", f"D_{tag}")
        lp = self.tl("lp", f"lp_{tag}")
        w = self.tl("w", f"w_{tag}")
        ns = self.tl("ns", f"ns_{tag}")
        self.lap(q, dt_, lp)
        self.wrap2(w, q, 1.0, q, 0.0)
        self.sin(ns, w)
        self.stt("dve", ah, lp, self.g, ns)

    def ah_eval_fused(self, a, sa, b, ah, q_out, tag):
        """Position q = sa*a + b; emit q (for lap) plus fused wrap, then
        ah = g*lap(q) + ns.  q_out must be a fresh tile."""
        dt_ = self.tl("D", f"D_{tag}")
        lp = self.tl("lp", f"lp_{tag}")
        w = self.tl("w", f"w_{tag}")
        ns = self.tl("ns", f"ns_{tag}")
        self.stt("dve", q_out, a, sa, b)      # q materialized for shifts
        self.wrap2(w, a, sa, b, 1.0)          # frac(sa*a + b) w/o waiting q
        self.sin(ns, w)
        self.lap(q_out, dt_, lp)
        self.stt("dve", ah, lp, self.g, ns)

    def rkn4_step(self, th, u, th_new, u_new, ah1_out, tag):
        """One RKN4 step; ah1_out receives ah(theta_n) for SC history."""
        h, g = self.h, self.g
        kap = h * h / 8.0                      # a1s = (kap/2pi) * ah1
        lam = h * h / 2.0                      # a2s = (lam/2pi) * ah2
        self.ah_eval(th, ah1_out, f"e1_{tag}")
        p2a = self.tl("p2a", f"p2a_{tag}")
        self.stt("dve", p2a, u, h / 2.0, th)
        q2 = self.tl("q2", f"q2_{tag}")
        ah2 = self.tl("ah2", f"ah2_{tag}")
        self.ah_eval_fused(ah1_out, kap / TWO_PI, p2a, ah2, q2,
                           f"e2_{tag}")
        t = self.tl("t", f"t_{tag}")
        self.stt("dve", t, u, h, th)
        q3 = self.tl("q3", f"q3_{tag}")
        ah3 = self.tl("ah3", f"ah3_{tag}")
        self.ah_eval_fused(ah2, lam / TWO_PI, t, ah3, q3, f"e3_{tag}")
        # theta' = t + (kap/2pi)*(4/3)*(ah1 + 2 ah2)
        g2 = self.tl("g2", f"g2_{tag}")
        self.stt("dve", g2, ah2, 2.0, ah1_out)
        self.stt("dve", th_new, g2, (4.0 / 3.0) * kap / TWO_PI, t)
        # u' = u + (h/(12pi)) * (ah1 + 4 ah2 + ah3)
        z1 = self.tl("z1", f"z1_{tag}")
        self.stt("dve", z1, ah2, 4.0, ah1_out)
        z2 = self.tl("z2", f"z2_{tag}")
        self.add_gps(z2, z1, ah3)
        self.stt("dve", u_new, z2, h / (12.0 * math.pi), u)

    def sc6_step(self, th, Z, th_new, Z_new, hist, ah_out, tag):
        """One Cowell step. hist = [ah_{n-1},...,ah_{n-4}] (newest first);
        ah_out receives ah(theta_n).
        theta' = theta + Z'; Z' = Z + (h^2/2pi)*sum beta_j ah_{n-j}."""
        h = self.h
        # Emission order = engine queue order (in-order execution): put the
        # critical chain's DVE ops (wrap) first; the history Horner fills
        # DVE's wait-for-sin gap.
        s1 = self.tl("s1", f"s1_{tag}")
        s2 = self.tl("s2", f"s2_{tag}")
        s3 = self.tl("s3", f"s3_{tag}")
        s4 = self.tl("s4", f"s4_{tag}")
        dt_ = self.tl("D", "D_cw")
        lp = self.tl("lp", "lp_cw")
        w = self.tl("w", "w_cw")
        ns = self.tl("ns", "ns_cw")
        self.wrap2(w, th, 1.0, th, 0.0)       # DVE: critical, first
        self.sin(ns, w)                       # ACT
        self.lap(th, dt_, lp)                 # GPS mains + DVE tinies
        self.stt_full(s1, hist[3], BETA[4] / BETA[3], hist[2])
        self.stt_full(s2, s1, BETA[3] / BETA[2], hist[1])
        self.stt_full(s3, s2, BETA[2] / BETA[1], hist[0])
        self.stt("dve", ah_out, lp, self.g, ns)
        self.stt("dve", s4, s3, BETA[1] / BETA[0], ah_out)
        self.stt("dve", Z_new, s4, BETA[0] * h * h / TWO_PI, Z)
        self.add_gps(th_new, th, Z_new)


def _build(nsteps: int, omega0: float, coupling: float) -> bass.Bass:
    tau_end = omega0 * T_END
    h = tau_end / nsteps
    cp = coupling / (omega0 * omega0)

    nc = bacc.Bacc("TRN2", target_bir_lowering=False, debug=False,
                   num_devices=N_CORES)
    x_in = nc.dram_tensor("x", [128, N], F32, kind="ExternalInput")
    out = nc.dram_tensor("out", [128, N], F32, kind="ExternalOutput")

    with tile.TileContext(nc) as tc:
        with (
            tc.tile_pool(name="state", bufs=1) as state,
            tc.tile_pool(name="tmp", bufs=2) as tmp,
        ):
            st = _Steps(nc, tmp, h, cp)

            xs = state.tile([128, N], F32, name="xs")
            nc.gpsimd.dma_start(xs[:], x_in[:])
            th = [state.tile([128, N], F32, name=f"th{i}") for i in range(2)]
            u = [state.tile([128, N], F32, name=f"u{i}") for i in range(2)]
            Z = [state.tile([128, N], F32, name=f"Z{i}") for i in range(2)]
            hist = [state.tile([128, N], F32, name=f"ah{i}")
                    for i in range(NHIST)]

            # init: theta = x - 0.5 (turns); u = 0
            nc.scalar.activation(th[0][:], xs[:], AF.Copy, bias=-0.5,
                                 scale=1.0)
            nc.vector.memset(u[0][:], 0.0)

            n_sc = nsteps - NSTART
            assert n_sc >= 1
            # startup: RKN4 steps, saving ah(theta_n) into history
            # hist order after startup (newest first for step NSTART):
            #   [ah_{NSTART-1}, ..., ah_0]
            cur, nxt = 0, 1
            prev_th = None
            for n in range(NSTART):
                st.rkn4_step(th[cur], u[cur], th[nxt], u[nxt],
                             hist[NSTART - 1 - n], "su")
                prev_th = th[cur]
                cur, nxt = nxt, cur
            # Z = theta_NSTART - theta_{NSTART-1}
            nc.vector.tensor_tensor(Z[0][:], th[cur][:], prev_th[:],
                                    OP.subtract)

            zc, zn = 0, 1
            hlist = list(hist[:NSTART])       # newest first
            for n in range(n_sc):
                # write into the slot that just fell out of the history
                ah_new = hist[(NHIST - 1 - (n % NHIST)) % NHIST]
                st.sc6_step(th[cur], Z[zc], th[nxt], Z[zn], hlist, ah_new,
                            "cw")
                hlist = [ah_new] + hlist[:NSTART - 1]
                cur, nxt = nxt, cur
                zc, zn = zn, zc

            # out = 2pi * theta  (radians)
            rad = tmp.tile([128, N], F32, name="rad", tag="rad")
            nc.scalar.activation(rad[:], th[cur][:], AF.Copy, bias=0.0,
                                 scale=TWO_PI)
            nc.gpsimd.dma_start(out[:], rad[:])

    nc.compile()
    return nc


_CACHE: dict = {}


def kernel(x, omega0, coupling, nsteps: int = None):
    x = np.ascontiguousarray(np.asarray(x, dtype=np.float32))
    om = float(np.asarray(omega0, dtype=np.float64))
    cp = float(np.asarray(coupling, dtype=np.float64))
    if nsteps is None:
        nsteps = NSTEPS
    key = (nsteps, om, cp)
    if key not in _CACHE:
        _CACHE[key] = _build(nsteps, om, cp)
    nc = _CACHE[key]

    in_maps = [{"x": x[i * PB:(i + 1) * PB]} for i in range(N_CORES)]
    res = run_bass_kernel_spmd(nc, in_maps, list(range(N_CORES)))
    return np.concatenate([r["out"] for r in res.results],
                          axis=0).astype(np.float32)
